# revision 6
# baseline (speedup 1.0000x reference)
"""GraphSpectralFilterLayer Trainium2 kernel.

Computes, for full inputs x[8192,512], W[64,512], attention_raw[16384,8192], k=32:
  h   = x @ W.T                                   [8192, 64]
  att = softmax(topk-mask(attention_raw), axis=1) [16384, 8192]  (k-sparse rows)
  h'  = att @ h                                   [16384, 64]
  out = h'.reshape(2, 8192, 64).transpose(1,0,2).reshape(8192, 128)
Returns (out, att) like the reference.

Sharding: attention rows split across 8 NeuronCores (2048 rows each); x, W
replicated; each core computes its att slice + h' slice; host concatenates.

Per-core algorithm, per 128-row tile:
  - segmented top-8 (32 segments x 256) -> 256 candidates   [DVE max8]
  - 4 rounds max8 + match_replace on candidates -> top-32 values,
    row max m, threshold t = 32nd largest
  - Z = sum exp(top32 - m) via ACT accum; bias = -(m + ln Z)
  - mask pass: f = (a < t) * -400  (+ per-row selected count via accum_out)
  - att = Exp(a + f + bias)  -> masked entries underflow to exactly 0
  - spmm: PE-transpose att chunks, matmul against resident h
Exact fp32 ties at the k-boundary (count != 32, data-dependent, detected via
the device-emitted counts) are fixed up generically on the host.
"""

import sys

for _p in ("/opt/pypackages", "/opt/trn_rl_repo"):
    if _p not in sys.path:
        sys.path.insert(0, _p)

from contextlib import ExitStack

import numpy as np

import concourse.bacc as bacc
import concourse.mybir as mybir
import concourse.tile as tile
from concourse.masks import make_identity
from concourse.bass_utils import run_bass_kernel_spmd

FP32 = mybir.dt.float32
AF = mybir.ActivationFunctionType
OP = mybir.AluOpType

N = 8192          # nodes / attention row length
IN_F = 512
OUT_F = 64
C = 2
K = 32
NCORES = 8
ROWS = C * N // NCORES   # 2048 attention rows per core
P = 128                  # partitions
SEG = 256                # top-8 segment width
NSEG = N // SEG          # 32 segments
NCH = N // P             # 64 node chunks
NEG_FILL = -400.0        # exp(a - m + NEG_FILL-ish) == 0.0 in fp32
SENTINEL = -1.0e30


def build_program(rows=ROWS):
    nc = bacc.Bacc("TRN2", target_bir_lowering=False, debug=False)
    araw = nc.dram_tensor("araw", [rows, N], FP32, kind="ExternalInput")
    x_d = nc.dram_tensor("x", [N, IN_F], FP32, kind="ExternalInput")
    w_d = nc.dram_tensor("w", [OUT_F, IN_F], FP32, kind="ExternalInput")
    att_d = nc.dram_tensor("att", [rows, N], FP32, kind="ExternalOutput")
    hp_d = nc.dram_tensor("hp", [rows, OUT_F], FP32, kind="ExternalOutput")
    cnt_d = nc.dram_tensor("cnt", [rows, 1], FP32, kind="ExternalOutput")

    ntiles = rows // P

    with tile.TileContext(nc) as tc, ExitStack() as ctx:
        big = ctx.enter_context(tc.tile_pool(name="big", bufs=2))
        work = ctx.enter_context(tc.tile_pool(name="work", bufs=2))
        small = ctx.enter_context(tc.tile_pool(name="small", bufs=2))
        persist = ctx.enter_context(tc.tile_pool(name="persist", bufs=1))
        attTp = ctx.enter_context(tc.tile_pool(name="attT", bufs=3))
        psum_t = ctx.enter_context(tc.tile_pool(name="psum_t", bufs=2, space="PSUM"))
        psum_mm = ctx.enter_context(tc.tile_pool(name="psum_mm", bufs=2, space="PSUM"))

        ident = persist.tile([P, P], FP32)
        make_identity(nc, ident)

        # ---- W^T: wT[:, j*64:(j+1)*64] = W[:, j*128:(j+1)*128].T ----
        w_sb = persist.tile([OUT_F, IN_F], FP32)
        nc.sync.dma_start(w_sb, w_d[:, :])
        wT = persist.tile([P, (IN_F // P) * OUT_F], FP32)
        for j in range(IN_F // P):
            tpw = psum_t.tile([P, 4 * P], FP32, tag="tp")
            tp = tpw[:, :OUT_F]
            nc.tensor.transpose(tp, w_sb[:, j * P:(j + 1) * P], ident[:OUT_F, :OUT_F])
            nc.scalar.copy(wT[:, j * OUT_F:(j + 1) * OUT_F], tp)

        # ---- h = x @ W.T, chunk-major resident: h_sb[:, n*64:(n+1)*64] = h[n*128:(n+1)*128, :] ----
        h_sb = persist.tile([P, NCH * OUT_F], FP32)
        for n in range(NCH):
            xc = small.tile([P, IN_F], FP32, tag="xc")
            nc.sync.dma_start(xc, x_d[n * P:(n + 1) * P, :])
            hps = psum_mm.tile([P, OUT_F], FP32, tag="acc")
            for j in range(IN_F // P):
                xTf = psum_t.tile([P, 4 * P], FP32, tag="tp")
                xT = xTf[:, :P]
                nc.tensor.transpose(xT, xc[:, j * P:(j + 1) * P], ident)
                xTs = small.tile([P, P], FP32, tag="xTs")
                nc.vector.tensor_copy(xTs, xT)
                nc.tensor.matmul(hps, lhsT=xTs, rhs=wT[:, j * OUT_F:(j + 1) * OUT_F],
                                 start=(j == 0), stop=(j == IN_F // P - 1))
            nc.scalar.copy(h_sb[:, n * OUT_F:(n + 1) * OUT_F], hps)

        # ---- main loop over row tiles ----
        for i in range(ntiles):
            a = big.tile([P, N], FP32, tag="a")
            nc.sync.dma_start(a, araw[i * P:(i + 1) * P, :])

            # segmented top-8 -> candidates
            cand = small.tile([P, NSEG * 8], FP32, tag="cand")
            for s in range(NSEG):
                nc.vector.max(cand[:, s * 8:(s + 1) * 8], a[:, s * SEG:(s + 1) * SEG])

            # 4 rounds of max8 (+match_replace) -> top-32 values, sorted per round
            vals = small.tile([P, K], FP32, tag="vals")
            nc.vector.max(vals[:, 0:8], cand)
            for r in range(1, K // 8):
                nc.vector.match_replace(out=cand, in_to_replace=vals[:, (r - 1) * 8:r * 8],
                                        in_values=cand, imm_value=SENTINEL)
                nc.vector.max(vals[:, r * 8:(r + 1) * 8], cand)
            m_ap = vals[:, 0:1]
            t_ap = vals[:, K - 1:K]

            # Z and final exp bias = -(m + lnZ)
            negm = small.tile([P, 1], FP32, tag="negm")
            nc.vector.tensor_scalar_mul(negm, m_ap, -1.0)
            ex32 = small.tile([P, K], FP32, tag="ex32")
            zrow = small.tile([P, 1], FP32, tag="zrow")
            nc.scalar.activation(ex32, vals, AF.Exp, bias=negm, scale=1.0, accum_out=zrow)
            lnz = small.tile([P, 1], FP32, tag="lnz")
            nc.scalar.activation(lnz, zrow, AF.Ln)
            biasf = small.tile([P, 1], FP32, tag="biasf")
            nc.vector.tensor_scalar(biasf, lnz, m_ap, -1.0, op0=OP.add, op1=OP.mult)

            # selected count = 24 + #(cand_after >= t); all entries >= t are
            # candidates, and rounds 1-3 removed exactly 24 of them
            acc = small.tile([P, 1], FP32, tag="acc")
            cmask = small.tile([P, NSEG * 8], FP32, tag="cmask")
            nc.vector.tensor_scalar(cmask, cand, t_ap, None, op0=OP.is_ge, op1=OP.add,
                                    accum_out=acc)
            nc.sync.dma_start(cnt_d[i * P:(i + 1) * P, :], acc)

            # mask pass: f = (a < t) * -400
            f = work.tile([P, N], FP32, tag="f")
            nc.vector.tensor_scalar(f, a, t_ap, NEG_FILL, op0=OP.is_lt, op1=OP.mult)
            nc.gpsimd.tensor_add(f, f, a)
            nc.scalar.activation(f, f, AF.Exp, bias=biasf, scale=1.0)
            nc.sync.dma_start(att_d[i * P:(i + 1) * P, :], f)

            # spmm: h'[tile] = att_tile @ h
            hps2 = psum_mm.tile([P, OUT_F], FP32, tag="acc")
            for g in range(NCH // 4):
                tp = psum_t.tile([P, 4 * P], FP32, tag="tp")
                for q in range(4):
                    c = g * 4 + q
                    nc.tensor.transpose(tp[:, q * P:(q + 1) * P], f[:, c * P:(c + 1) * P], ident)
                aT = attTp.tile([P, 4 * P], FP32, tag="aT")
                nc.vector.tensor_copy(aT, tp)
                for q in range(4):
                    c = g * 4 + q
                    nc.tensor.matmul(hps2, lhsT=aT[:, q * P:(q + 1) * P],
                                     rhs=h_sb[:, c * OUT_F:(c + 1) * OUT_F],
                                     start=(c == 0), stop=(c == NCH - 1))
            hpt = small.tile([P, OUT_F], FP32, tag="hpt")
            nc.scalar.copy(hpt, hps2)
            nc.sync.dma_start(hp_d[i * P:(i + 1) * P, :], hpt)

    nc.finalize()
    return nc


_CACHED = {}


def _get_program(rows=ROWS):
    if rows not in _CACHED:
        _CACHED[rows] = build_program(rows)
    return _CACHED[rows]


def _fix_ties(att, hp, counts, attention_raw, x, W):
    """Zero the later-index duplicates at the top-k fp32 tie boundary so the
    selection matches topk (lowest index wins); adjust h' accordingly."""
    bad = np.flatnonzero(counts != K)
    for r in bad:
        row = attention_raw[r]
        t = np.partition(row, N - K)[N - K]
        pos = np.flatnonzero(row == t)
        keep = K - int((row > t).sum())
        for p in pos[keep:]:
            v = att[r, p]
            att[r, p] = 0.0
            hp[r] -= v * (x[p] @ W.T)
    return att, hp


def kernel(x, W, attention_raw, k):
    assert int(k) == K
    x = np.ascontiguousarray(x, dtype=np.float32)
    W = np.ascontiguousarray(W, dtype=np.float32)
    attention_raw = np.ascontiguousarray(attention_raw, dtype=np.float32)
    assert attention_raw.shape == (C * N, N)

    nc = _get_program()
    in_maps = []
    for i in range(NCORES):
        in_maps.append({
            "araw": attention_raw[i * ROWS:(i + 1) * ROWS],
            "x": x,
            "w": W,
        })
    res = run_bass_kernel_spmd(nc, in_maps, core_ids=list(range(NCORES)))
    results = res.results

    att = np.concatenate([r["att"] for r in results], axis=0)
    hp = np.concatenate([r["hp"] for r in results], axis=0)
    acc = np.concatenate([r["cnt"] for r in results], axis=0)[:, 0].astype(np.float64)
    counts = np.rint(acc + (K - 8)).astype(np.int64)

    att, hp = _fix_ties(att, hp, counts, attention_raw, x, W)

    out = hp.reshape(C, N, OUT_F).transpose(1, 0, 2).reshape(N, C * OUT_F)
    return np.ascontiguousarray(out), att


# revision 7
# speedup vs baseline: 1.3378x; 1.3378x over previous
"""GraphSpectralFilterLayer Trainium2 kernel.

Computes, for full inputs x[8192,512], W[64,512], attention_raw[16384,8192], k=32:
  h   = x @ W.T                                   [8192, 64]
  att = softmax(topk-mask(attention_raw), axis=1) [16384, 8192]  (k-sparse rows)
  h'  = att @ h                                   [16384, 64]
  out = h'.reshape(2, 8192, 64).transpose(1,0,2).reshape(8192, 128)
Returns (out, att) like the reference.

Sharding: attention rows split across 8 NeuronCores (2048 rows each); x, W
replicated; each core computes its att slice + h' slice; host concatenates.

Per-core algorithm, per 128-row tile:
  - segmented top-8 (32 segments x 256) -> 256 candidates   [DVE max8]
  - 4 rounds max8 + match_replace on candidates -> top-32 values,
    row max m, threshold t = 32nd largest
  - Z = sum exp(top32 - m) via ACT accum; bias = -(m + ln Z)
  - mask pass: f = (a < t) * -400  (+ per-row selected count via accum_out)
  - att = Exp(a + f + bias)  -> masked entries underflow to exactly 0
  - spmm: PE-transpose att chunks, matmul against resident h
Exact fp32 ties at the k-boundary (count != 32, data-dependent, detected via
the device-emitted counts) are fixed up generically on the host.
"""

import sys

for _p in ("/opt/pypackages", "/opt/trn_rl_repo"):
    if _p not in sys.path:
        sys.path.insert(0, _p)

from contextlib import ExitStack

import numpy as np

import concourse.bacc as bacc
import concourse.mybir as mybir
import concourse.tile as tile
from concourse.masks import make_identity
from concourse.bass_utils import run_bass_kernel_spmd

FP32 = mybir.dt.float32
AF = mybir.ActivationFunctionType
OP = mybir.AluOpType

N = 8192          # nodes / attention row length
IN_F = 512
OUT_F = 64
C = 2
K = 32
NCORES = 8
ROWS = C * N // NCORES   # 2048 attention rows per core
P = 128                  # partitions
SEG = 256                # top-8 segment width
NSEG = N // SEG          # 32 segments
NCH = N // P             # 64 node chunks
NEG_FILL = -400.0        # exp(a - m + NEG_FILL-ish) == 0.0 in fp32
SENTINEL = -1.0e30


def build_program(rows=ROWS):
    nc = bacc.Bacc("TRN2", target_bir_lowering=False, debug=False)
    araw = nc.dram_tensor("araw", [rows, N], FP32, kind="ExternalInput")
    x_d = nc.dram_tensor("x", [N, IN_F], FP32, kind="ExternalInput")
    w_d = nc.dram_tensor("w", [OUT_F, IN_F], FP32, kind="ExternalInput")
    att_d = nc.dram_tensor("att", [rows, N], FP32, kind="ExternalOutput")
    hp_d = nc.dram_tensor("hp", [rows, OUT_F], FP32, kind="ExternalOutput")
    cnt_d = nc.dram_tensor("cnt", [rows, 1], FP32, kind="ExternalOutput")

    ntiles = rows // P

    with tile.TileContext(nc) as tc, ExitStack() as ctx:
        big = ctx.enter_context(tc.tile_pool(name="big", bufs=2))
        work = ctx.enter_context(tc.tile_pool(name="work", bufs=2))
        small = ctx.enter_context(tc.tile_pool(name="small", bufs=2))
        persist = ctx.enter_context(tc.tile_pool(name="persist", bufs=1))
        attTp = ctx.enter_context(tc.tile_pool(name="attT", bufs=4))
        psum_t = ctx.enter_context(tc.tile_pool(name="psum_t", bufs=4, space="PSUM"))
        psum_mm = ctx.enter_context(tc.tile_pool(name="psum_mm", bufs=2, space="PSUM"))

        ident = persist.tile([P, P], FP32)
        make_identity(nc, ident)

        # ---- W^T: wT[:, j*64:(j+1)*64] = W[:, j*128:(j+1)*128].T ----
        w_sb = persist.tile([OUT_F, IN_F], FP32)
        nc.sync.dma_start(w_sb, w_d[:, :])
        wT = persist.tile([P, (IN_F // P) * OUT_F], FP32)
        for j in range(IN_F // P):
            tpw = psum_t.tile([P, 4 * P], FP32, tag="tp")
            tp = tpw[:, :OUT_F]
            nc.tensor.transpose(tp, w_sb[:, j * P:(j + 1) * P], ident[:OUT_F, :OUT_F])
            nc.scalar.copy(wT[:, j * OUT_F:(j + 1) * OUT_F], tp)

        # ---- h = x @ W.T, chunk-major resident: h_sb[:, n*64:(n+1)*64] = h[n*128:(n+1)*128, :] ----
        h_sb = persist.tile([P, NCH * OUT_F], FP32)
        for n in range(NCH):
            xc = small.tile([P, IN_F], FP32, tag="xc")
            nc.sync.dma_start(xc, x_d[n * P:(n + 1) * P, :])
            hps = psum_mm.tile([P, OUT_F], FP32, tag="acc")
            for j in range(IN_F // P):
                xTf = psum_t.tile([P, 4 * P], FP32, tag="tp")
                xT = xTf[:, :P]
                nc.tensor.transpose(xT, xc[:, j * P:(j + 1) * P], ident)
                xTs = small.tile([P, P], FP32, tag="xTs")
                nc.scalar.copy(xTs, xT)
                nc.tensor.matmul(hps, lhsT=xTs, rhs=wT[:, j * OUT_F:(j + 1) * OUT_F],
                                 start=(j == 0), stop=(j == IN_F // P - 1))
            nc.scalar.copy(h_sb[:, n * OUT_F:(n + 1) * OUT_F], hps)

        # ---- main loop over row tiles ----
        for i in range(ntiles):
            a = big.tile([P, N], FP32, tag="a")
            nc.sync.dma_start(a, araw[i * P:(i + 1) * P, :])

            # segmented top-8 -> candidates
            cand = small.tile([P, NSEG * 8], FP32, tag="cand")
            for s in range(NSEG):
                nc.vector.max(cand[:, s * 8:(s + 1) * 8], a[:, s * SEG:(s + 1) * SEG])

            # 4 rounds of max8 (+match_replace) -> top-32 values, sorted per round
            vals = small.tile([P, K], FP32, tag="vals")
            nc.vector.max(vals[:, 0:8], cand)
            for r in range(1, K // 8):
                nc.vector.match_replace(out=cand, in_to_replace=vals[:, (r - 1) * 8:r * 8],
                                        in_values=cand, imm_value=SENTINEL)
                nc.vector.max(vals[:, r * 8:(r + 1) * 8], cand)
            m_ap = vals[:, 0:1]
            t_ap = vals[:, K - 1:K]

            # Z and final exp bias = -(m + lnZ)
            negm = small.tile([P, 1], FP32, tag="negm")
            nc.vector.tensor_scalar_mul(negm, m_ap, -1.0)
            ex32 = small.tile([P, K], FP32, tag="ex32")
            zrow = small.tile([P, 1], FP32, tag="zrow")
            nc.scalar.activation(ex32, vals, AF.Exp, bias=negm, scale=1.0, accum_out=zrow)
            lnz = small.tile([P, 1], FP32, tag="lnz")
            nc.scalar.activation(lnz, zrow, AF.Ln)
            biasf = small.tile([P, 1], FP32, tag="biasf")
            nc.vector.tensor_scalar(biasf, lnz, m_ap, -1.0, op0=OP.add, op1=OP.mult)

            # selected count = 24 + #(cand_after >= t); all entries >= t are
            # candidates, and rounds 1-3 removed exactly 24 of them
            acc = small.tile([P, 1], FP32, tag="acc")
            cmask = small.tile([P, NSEG * 8], FP32, tag="cmask")
            nc.vector.tensor_scalar(cmask, cand, t_ap, None, op0=OP.is_ge, op1=OP.add,
                                    accum_out=acc)
            nc.sync.dma_start(cnt_d[i * P:(i + 1) * P, :], acc)

            # mask pass: f = (a < t) * -400
            f = work.tile([P, N], FP32, tag="f")
            nc.vector.tensor_scalar(f, a, t_ap, NEG_FILL, op0=OP.is_lt, op1=OP.mult)
            nc.gpsimd.tensor_add(f, f, a)
            nc.scalar.activation(f, f, AF.Exp, bias=biasf, scale=1.0)
            nc.sync.dma_start(att_d[i * P:(i + 1) * P, :], f)

            # spmm: h'[tile] = att_tile @ h
            hps2 = psum_mm.tile([P, OUT_F], FP32, tag="acc")
            for g in range(NCH // 4):
                tp = psum_t.tile([P, 4 * P], FP32, tag="tp")
                for q in range(4):
                    c = g * 4 + q
                    nc.tensor.transpose(tp[:, q * P:(q + 1) * P], f[:, c * P:(c + 1) * P], ident)
                aT = attTp.tile([P, 4 * P], FP32, tag="aT")
                nc.scalar.copy(aT, tp)
                for q in range(4):
                    c = g * 4 + q
                    nc.tensor.matmul(hps2, lhsT=aT[:, q * P:(q + 1) * P],
                                     rhs=h_sb[:, c * OUT_F:(c + 1) * OUT_F],
                                     start=(c == 0), stop=(c == NCH - 1))
            hpt = small.tile([P, OUT_F], FP32, tag="hpt")
            nc.scalar.copy(hpt, hps2)
            nc.sync.dma_start(hp_d[i * P:(i + 1) * P, :], hpt)

    nc.finalize()
    return nc


_CACHED = {}


def _get_program(rows=ROWS):
    if rows not in _CACHED:
        _CACHED[rows] = build_program(rows)
    return _CACHED[rows]


def _fix_ties(att, hp, counts, attention_raw, x, W):
    """Zero the later-index duplicates at the top-k fp32 tie boundary so the
    selection matches topk (lowest index wins); adjust h' accordingly."""
    bad = np.flatnonzero(counts != K)
    for r in bad:
        row = attention_raw[r]
        t = np.partition(row, N - K)[N - K]
        pos = np.flatnonzero(row == t)
        keep = K - int((row > t).sum())
        for p in pos[keep:]:
            v = att[r, p]
            att[r, p] = 0.0
            hp[r] -= v * (x[p] @ W.T)
    return att, hp


def kernel(x, W, attention_raw, k):
    assert int(k) == K
    x = np.ascontiguousarray(x, dtype=np.float32)
    W = np.ascontiguousarray(W, dtype=np.float32)
    attention_raw = np.ascontiguousarray(attention_raw, dtype=np.float32)
    assert attention_raw.shape == (C * N, N)

    nc = _get_program()
    in_maps = []
    for i in range(NCORES):
        in_maps.append({
            "araw": attention_raw[i * ROWS:(i + 1) * ROWS],
            "x": x,
            "w": W,
        })
    res = run_bass_kernel_spmd(nc, in_maps, core_ids=list(range(NCORES)))
    results = res.results

    att = np.concatenate([r["att"] for r in results], axis=0)
    hp = np.concatenate([r["hp"] for r in results], axis=0)
    acc = np.concatenate([r["cnt"] for r in results], axis=0)[:, 0].astype(np.float64)
    counts = np.rint(acc + (K - 8)).astype(np.int64)

    att, hp = _fix_ties(att, hp, counts, attention_raw, x, W)

    out = hp.reshape(C, N, OUT_F).transpose(1, 0, 2).reshape(N, C * OUT_F)
    return np.ascontiguousarray(out), att
